# revision 10
# baseline (speedup 1.0000x reference)
"""MoE ConvNeXt block (dwconv7x7 -> LN -> top2-of-8 MoE MLP -> layerscale residual)
on 8 trn2 NeuronCores, data-parallel over the batch dim (4 images per core).

Layout strategy: channel-major [C on partitions (3 chunks of 128), tokens on free].
 - dwconv: 49 diagonal-stationary matmuls accumulating in PSUM (per tap, shifted AP
   into an h/w zero-padded input buffer).
 - LN: column sums via ones-stationary matmuls (replicated across partitions), fused
   scale/shift on DVE.
 - router: token-major logits via x-as-stationary matmuls -> [128 tok, 8] PSUM tiles;
   top-2 + softmax with DVE reduce/select ops.
 - MoE: dense (all 8 experts), weight-stationary matmuls; per-expert gate weights
   broadcast across partitions (DRAM bounce + gpsimd partition_broadcast) and applied
   to the expert output before accumulation.
"""

import sys

sys.path.insert(0, "/opt/trn_rl_repo/concourse")
sys.path.insert(0, "/opt/trn_rl_repo")

import numpy as np
import ml_dtypes

import concourse.bass as bass
import concourse.tile as tile
from concourse import bacc, mybir
from concourse import bass_utils

F32 = mybir.dt.float32
BF16 = mybir.dt.bfloat16
AF = mybir.ActivationFunctionType
OP = mybir.AluOpType

DIM = 384
NE = 8
HID = 4 * DIM  # 1536
NIMG = 4  # images per core
T = NIMG * 1024  # tokens per core
NQ = 3  # channel chunks of 128
NHT = HID // 128  # 12
NCB = 8  # 512-token column blocks
CB = 512
EPS = 1e-6

_cached = None


def _build():
    nc = bacc.Bacc("TRN2", target_bir_lowering=False)

    inp4 = nc.dram_tensor("inp4", [NIMG, DIM, 32, 32], F32, kind="ExternalInput")
    diag = nc.dram_tensor("diag", [NQ, 49, 128, 128], BF16, kind="ExternalInput")
    w1d = nc.dram_tensor("w1d", [NE, NQ, 128, HID], BF16, kind="ExternalInput")
    w2d = nc.dram_tensor("w2d", [NE, NHT, 128, DIM], BF16, kind="ExternalInput")
    b1s = nc.dram_tensor("b1s", [128, NE, NHT], F32, kind="ExternalInput")
    b2s = nc.dram_tensor("b2s", [128, NE, NQ], F32, kind="ExternalInput")
    gws = nc.dram_tensor("gws", [NQ, 128, NE], BF16, kind="ExternalInput")
    chv = nc.dram_tensor("chv", [128, NQ, 4], F32, kind="ExternalInput")
    io8 = nc.dram_tensor("io8", [128, NE], F32, kind="ExternalInput")
    out4 = nc.dram_tensor("out4", [NIMG, DIM, 32, 32], F32, kind="ExternalOutput")

    inp_cm = inp4.rearrange("n c h w -> c n (h w)")  # [384, 4, 1024]
    out_cm = out4.rearrange("n c h w -> c n (h w)")

    with tile.TileContext(nc) as tc:
        # ---------- persistent SBUF ----------
        persist = tc.alloc_tile_pool(name="persist", bufs=1)
        xhat = [persist.tile([128, T], BF16, tag=f"xhat{q}", name=f"xhat{q}") for q in range(NQ)]
        acc = [persist.tile([128, T], BF16, tag=f"acc{q}", name=f"acc{q}") for q in range(NQ)]
        b1t = persist.tile([128, NE, NHT], F32, tag="b1t", name="b1t")
        b2t = persist.tile([128, NE, NQ], F32, tag="b2t", name="b2t")
        gwt = persist.tile([128, NQ, NE], BF16, tag="gwt", name="gwt")
        chvt = persist.tile([128, NQ, 4], F32, tag="chvt", name="chvt")
        io8t = persist.tile([128, NE], F32, tag="io8t", name="io8t")
        onest = persist.tile([128, 128], BF16, tag="onest", name="onest")
        m1v = persist.tile([128, 32], F32, tag="m1v", name="m1v")
        m2v = persist.tile([128, 32], F32, tag="m2v", name="m2v")
        e0v = persist.tile([128, 32], F32, tag="e0v", name="e0v")
        e1v = persist.tile([128, 32], F32, tag="e1v", name="e1v")
        w0v = persist.tile([128, 32], F32, tag="w0v", name="w0v")
        w1v = persist.tile([128, 32], F32, tag="w1v", name="w1v")

        nc.sync.dma_start(b1t[:], b1s[:])
        nc.sync.dma_start(b2t[:], b2s[:])
        nc.sync.dma_start(gwt[:], gws.rearrange("q p e -> p q e"))
        nc.sync.dma_start(chvt[:], chv[:])
        nc.sync.dma_start(io8t[:], io8[:])
        nc.any.memset(onest[:], 1.0)
        epst = persist.tile([128, 1], F32, tag="epst", name="epst")
        nc.any.memset(epst[:], EPS)
        zerot = persist.tile([128, 1], F32, tag="zerot", name="zerot")
        nc.any.memset(zerot[:], 0.0)

        # ---------- phase 1: dwconv + LN stats inputs ----------
        with tc.tile_pool(name="convin", bufs=2) as cpool, \
             tc.tile_pool(name="diagp", bufs=1) as dpool, \
             tc.tile_pool(name="xconv", bufs=1) as xcpool, \
             tc.tile_pool(name="cps", bufs=4, space="PSUM") as cps, \
             tc.tile_pool(name="sps", bufs=2, space="PSUM") as sps, \
             tc.tile_pool(name="lnt", bufs=2) as lnt:
            xconv = [xcpool.tile([128, T], BF16, tag=f"xc{q}", name=f"xc{q}") for q in range(NQ)]
            for q in range(NQ):
                xp = cpool.tile([128, NIMG, 38, 38], BF16, tag="xpad", name="xpad")
                nc.any.memset(xp[:], 0.0)
                for n in range(NIMG):
                    nc.gpsimd.dma_start(
                        xp[:, n, 3:35, 3:35],
                        inp4.rearrange("n c h w -> c n h w")[q * 128:(q + 1) * 128, n],
                    )
                dg = dpool.tile([128, 49, 128], BF16, tag="diag", name="diag")
                nc.sync.dma_start(dg[:], diag.rearrange("q t p m -> p q t m")[:, q])
                for cbg in range(2):  # two groups of 4 column blocks
                    pts = [cps.tile([128, 16, 32], F32, tag="cpsum", name="cpsum") for _ in range(4)]
                    for tap in range(49):
                        dh, dw = tap // 7, tap % 7
                        for j in range(4):
                            cb = cbg * 4 + j
                            n, hh = cb // 2, cb % 2
                            nc.tensor.matmul(
                                pts[j][:],
                                dg[:, tap],
                                xp[:, n, hh * 16 + dh: hh * 16 + dh + 16, dw: dw + 32],
                                start=(tap == 0),
                                stop=(tap == 48),
                            )
                    for j in range(4):
                        cb = cbg * 4 + j
                        sl = slice(cb * CB, (cb + 1) * CB)
                        xcv = xconv[q][:, sl].rearrange("p (a b) -> p a b", a=16)
                        nc.scalar.activation(xcv, pts[j][:], AF.Identity,
                                             bias=chvt[:, q, 0:1], scale=1.0)

            # ---------- phase 2: LN stats + apply ----------
            for cb in range(NCB):
                sl = slice(cb * CB, (cb + 1) * CB)
                pm1 = sps.tile([128, CB], F32, tag="pm1", name="pm1")
                pm2 = sps.tile([128, CB], F32, tag="pm2", name="pm2")
                for q in range(NQ):
                    nc.tensor.matmul(pm1[:], onest[:], xconv[q][:, sl],
                                     start=(q == 0), stop=(q == NQ - 1))
                for q in range(NQ):
                    sqt = lnt.tile([128, CB], BF16, tag="sqt", name="sqt")
                    nc.scalar.activation(sqt[:], xconv[q][:, sl], AF.Square,
                                         bias=zerot[:], scale=1.0)
                    nc.tensor.matmul(pm2[:], onest[:], sqt[:],
                                     start=(q == 0), stop=(q == NQ - 1))
                mus = lnt.tile([128, CB], F32, tag="mus", name="mus")
                nc.vector.tensor_scalar_mul(mus[:], pm1[:], 1.0 / DIM)
                msq = lnt.tile([128, CB], F32, tag="msq", name="msq")
                nc.vector.tensor_tensor(msq[:], mus[:], mus[:], OP.mult)
                var = lnt.tile([128, CB], F32, tag="var", name="var")
                nc.vector.scalar_tensor_tensor(var[:], pm2[:], 1.0 / DIM, msq[:],
                                               OP.mult, OP.subtract)
                sd = lnt.tile([128, CB], F32, tag="sd", name="sd")
                nc.scalar.activation(sd[:], var[:], AF.Sqrt, bias=epst[:], scale=1.0)
                rst = lnt.tile([128, CB], F32, tag="rst", name="rst")
                nc.vector.reciprocal(rst[:], sd[:])
                for q in range(NQ):
                    t1 = lnt.tile([128, CB], F32, tag="t1", name="t1")
                    nc.vector.tensor_tensor(t1[:], xconv[q][:, sl], mus[:],
                                            OP.subtract)
                    t2 = lnt.tile([128, CB], F32, tag="t2", name="t2")
                    nc.vector.tensor_tensor(t2[:], t1[:], rst[:], OP.mult)
                    nc.vector.tensor_scalar(xhat[q][:, sl], t2[:],
                                            chvt[:, q, 1:2], chvt[:, q, 2:3],
                                            OP.mult, OP.add)

        # ---------- phase 3: router logits + top-2 ----------
        with tc.tile_pool(name="lps", bufs=4, space="PSUM") as lps, \
             tc.tile_pool(name="tkt", bufs=6) as tkt:
            for tt in range(32):
                plg = lps.tile([128, NE], F32, tag="plg", name="plg")
                for q in range(NQ):
                    nc.tensor.matmul(plg[:], xhat[q][:, tt * 128:(tt + 1) * 128],
                                     gwt[:, q], start=(q == 0), stop=(q == NQ - 1))
                c1 = slice(tt, tt + 1)
                nc.vector.tensor_reduce(m1v[:, c1], plg[:], mybir.AxisListType.X, OP.max)
                ta = tkt.tile([128, NE], F32, tag="ta", name="ta")
                nc.vector.tensor_scalar(ta[:], plg[:], m1v[:, c1], None, OP.is_equal)
                tb = tkt.tile([128, NE], F32, tag="tb", name="tb")
                nc.vector.tensor_tensor(tb[:], ta[:], io8t[:], OP.mult)
                nc.vector.tensor_reduce(e0v[:, c1], tb[:], mybir.AxisListType.X, OP.max)
                tcm = tkt.tile([128, NE], F32, tag="tc", name="tc")
                nc.vector.scalar_tensor_tensor(tcm[:], ta[:], -1e30, plg[:],
                                               OP.mult, OP.add)
                nc.vector.tensor_reduce(m2v[:, c1], tcm[:], mybir.AxisListType.X, OP.max)
                td = tkt.tile([128, NE], F32, tag="td", name="td")
                nc.vector.tensor_scalar(td[:], tcm[:], m2v[:, c1], None, OP.is_equal)
                te = tkt.tile([128, NE], F32, tag="te", name="te")
                nc.vector.tensor_tensor(te[:], td[:], io8t[:], OP.mult)
                nc.vector.tensor_reduce(e1v[:, c1], te[:], mybir.AxisListType.X, OP.max)
            # softmax over the two top values
            dv = tkt.tile([128, 32], F32, tag="dv", name="dv")
            nc.vector.tensor_tensor(dv[:], m2v[:], m1v[:], OP.subtract)
            ev = tkt.tile([128, 32], F32, tag="ev", name="ev")
            nc.scalar.activation(ev[:], dv[:], AF.Exp, bias=zerot[:], scale=1.0)
            den = tkt.tile([128, 32], F32, tag="den", name="den")
            nc.vector.tensor_scalar_add(den[:], ev[:], 1.0)
            nc.vector.reciprocal(w0v[:], den[:])
            nc.vector.tensor_scalar(w1v[:], w0v[:], -1.0, 1.0, OP.mult, OP.add)

        # ---------- phase 4: per-expert gate broadcast + dense MoE MLP ----------
        with tc.tile_pool(name="wd", bufs=1, space="DRAM") as wdp, \
             tc.tile_pool(name="wtok", bufs=4) as wtp, \
             tc.tile_pool(name="webp", bufs=2) as webp, \
             tc.tile_pool(name="wts", bufs=2) as wts, \
             tc.tile_pool(name="hsb", bufs=13) as hsb, \
             tc.tile_pool(name="hps", bufs=2, space="PSUM") as hps, \
             tc.tile_pool(name="yps", bufs=3, space="PSUM") as yps, \
             tc.tile_pool(name="cmb", bufs=3) as cmb:
            wd = wdp.tile([NE, 32, 128], BF16, name="wd")
            for e in range(NE):
                # gate weight for expert e per token, token-major [tok128, tile32]
                ma = wtp.tile([128, 32], F32, tag="ma", name="ma")
                nc.vector.tensor_scalar(ma[:], e0v[:], float(e), None, OP.is_equal)
                mb = wtp.tile([128, 32], F32, tag="mb", name="mb")
                nc.vector.tensor_tensor(mb[:], ma[:], w0v[:], OP.mult)
                nc.vector.tensor_scalar(ma[:], e1v[:], float(e), None, OP.is_equal)
                mc = wtp.tile([128, 32], F32, tag="mc", name="mc")
                nc.vector.tensor_tensor(mc[:], ma[:], w1v[:], OP.mult)
                wtok = wtp.tile([128, 32], BF16, tag="wtok", name="wtok")
                nc.vector.tensor_tensor(wtok[:], mb[:], mc[:], OP.add)
                nc.sync.dma_start(wd[e].rearrange("t p -> p t"), wtok[:])
                w1row = webp.tile([1, T], BF16, tag="w1row", name="w1row")
                nc.sync.dma_start(w1row[:], wd[e].rearrange("t p -> () (t p)"))
                web = webp.tile([128, T], BF16, tag="web", name="web")
                nc.gpsimd.partition_broadcast(web[:], w1row[:])

                w1t = wts.tile([128, NQ, HID], BF16, tag="w1t", name="w1t")
                nc.sync.dma_start(w1t[:], w1d.rearrange("e q p m -> p e q m")[:, e])
                w2t = wts.tile([128, NHT, DIM], BF16, tag="w2t", name="w2t")
                nc.sync.dma_start(w2t[:], w2d.rearrange("e t p m -> p e t m")[:, e])

                for cb in range(NCB):
                    sl = slice(cb * CB, (cb + 1) * CB)
                    hts = []
                    for ht in range(NHT):
                        ph = hps.tile([128, CB], F32, tag="ph", name="ph")
                        for q in range(NQ):
                            nc.tensor.matmul(ph[:],
                                             w1t[:, q, ht * 128:(ht + 1) * 128],
                                             xhat[q][:, sl],
                                             start=(q == 0), stop=(q == NQ - 1))
                        hgel = hsb.tile([128, CB], BF16, tag="hgel", name="hgel")
                        nc.scalar.activation(hgel[:], ph[:], AF.Gelu,
                                             bias=b1t[:, e, ht:ht + 1], scale=1.0)
                        hts.append(hgel)
                    for dq in range(NQ):
                        py = yps.tile([128, CB], F32, tag="py", name="py")
                        for ht in range(NHT):
                            nc.tensor.matmul(py[:],
                                             w2t[:, ht, dq * 128:(dq + 1) * 128],
                                             hts[ht][:],
                                             start=(ht == 0), stop=(ht == NHT - 1))
                        if e == 0:
                            nc.vector.scalar_tensor_tensor(
                                acc[dq][:, sl], py[:], b2t[:, e, dq:dq + 1],
                                web[:, sl], OP.add, OP.mult)
                        else:
                            ytmp = cmb.tile([128, CB], F32, tag="ytmp", name="ytmp")
                            nc.vector.scalar_tensor_tensor(
                                ytmp[:], py[:], b2t[:, e, dq:dq + 1],
                                web[:, sl], OP.add, OP.mult)
                            nc.vector.tensor_tensor(acc[dq][:, sl], acc[dq][:, sl],
                                                    ytmp[:], OP.add)

        # ---------- phase 5: layer-scale + residual + store ----------
        with tc.tile_pool(name="fin", bufs=3) as fin:
            for q in range(NQ):
                res = fin.tile([128, NIMG, 1024], F32, tag="res", name="res")
                nc.sync.dma_start(res[:], inp_cm[q * 128:(q + 1) * 128])
                osb = fin.tile([128, NIMG, 1024], F32, tag="osb", name="osb")
                nc.vector.scalar_tensor_tensor(
                    osb.rearrange("p n x -> p (n x)"), acc[q][:],
                    chvt[:, q, 3:4], res.rearrange("p n x -> p (n x)"),
                    OP.mult, OP.add)
                nc.sync.dma_start(out_cm[q * 128:(q + 1) * 128], osb[:])

        persist.release()

    nc.compile()
    return nc


def _prep(inputs):
    bf = ml_dtypes.bfloat16
    dw_w = np.asarray(inputs["dw_w"], np.float32)  # [384,1,7,7]
    diag = np.zeros((NQ, 49, 128, 128), np.float32)
    ii = np.arange(128)
    for q in range(NQ):
        for tap in range(49):
            diag[q, tap, ii, ii] = dw_w[q * 128:(q + 1) * 128, 0, tap // 7, tap % 7]
    w1 = np.asarray(inputs["w1"], np.float32)  # [8,384,1536]
    w2 = np.asarray(inputs["w2"], np.float32)  # [8,1536,384]
    w1d = w1.reshape(NE, NQ, 128, HID)
    w2d = w2.reshape(NE, NHT, 128, DIM)
    b1 = np.asarray(inputs["b1"], np.float32)  # [8,1536]
    b2 = np.asarray(inputs["b2"], np.float32)  # [8,384]
    b1s = b1.reshape(NE, NHT, 128).transpose(2, 0, 1)  # [128, 8, 12]
    b2s = b2.reshape(NE, NQ, 128).transpose(2, 0, 1)  # [128, 8, 3]
    gw = np.asarray(inputs["gate_w"], np.float32)  # [8,384]
    gws = gw.reshape(NE, NQ, 128).transpose(1, 2, 0)  # [3,128,8]
    chv = np.stack([
        np.asarray(inputs["dw_b"], np.float32),
        np.asarray(inputs["ln_g"], np.float32),
        np.asarray(inputs["ln_b"], np.float32),
        np.asarray(inputs["layer_scale"], np.float32).reshape(-1),
    ], axis=-1).reshape(NQ, 128, 4).transpose(1, 0, 2)  # [128,3,4]
    io8 = np.broadcast_to(np.arange(NE, dtype=np.float32), (128, NE))
    common = {
        "diag": np.ascontiguousarray(diag.astype(bf)),
        "w1d": np.ascontiguousarray(w1d.astype(bf)),
        "w2d": np.ascontiguousarray(w2d.astype(bf)),
        "b1s": np.ascontiguousarray(b1s),
        "b2s": np.ascontiguousarray(b2s),
        "gws": np.ascontiguousarray(gws.astype(bf)),
        "chv": np.ascontiguousarray(chv),
        "io8": np.ascontiguousarray(io8),
    }
    return common


def kernel(**inputs):
    global _cached
    if _cached is None:
        _cached = _build()
    nc = _cached
    common = _prep(inputs)
    inp = np.ascontiguousarray(np.asarray(inputs["input"], np.float32))
    in_maps = []
    for c in range(8):
        m = dict(common)
        m["inp4"] = np.ascontiguousarray(inp[c * NIMG:(c + 1) * NIMG])
        in_maps.append(m)
    res = bass_utils.run_bass_kernel_spmd(nc, in_maps, core_ids=list(range(8)))
    out = np.concatenate([res.results[c]["out4"] for c in range(8)], axis=0)
    return out.astype(np.float32)


if __name__ == "__main__":
    import reference
    inputs = {k: np.asarray(v) for k, v in reference.setup_inputs().items()}
    got = kernel(**inputs)
    exp = np.asarray(reference.reference(**reference.setup_inputs()))
    err = np.abs(got - exp)
    rel = err.max() / np.abs(exp).max()
    print("max abs err:", err.max(), "rel:", rel)


# revision 11
# speedup vs baseline: 6407.1743x; 6407.1743x over previous
"""MoE ConvNeXt block (dwconv7x7 -> LN -> top2-of-8 MoE MLP -> layerscale residual)
on 8 trn2 NeuronCores, data-parallel over the batch dim (4 images per core).

Layout strategy: channel-major [C on partitions (3 chunks of 128), tokens on free].
 - dwconv: 49 diagonal-stationary matmuls accumulating in PSUM (per tap, shifted AP
   into an h/w zero-padded input buffer).
 - LN: column sums via ones-stationary matmuls (replicated across partitions), fused
   scale/shift on DVE.
 - router: token-major logits via x-as-stationary matmuls -> [128 tok, 8] PSUM tiles;
   top-2 + softmax with DVE reduce/select ops.
 - MoE: dense (all 8 experts), weight-stationary matmuls; per-expert gate weights
   broadcast across partitions (DRAM bounce + gpsimd partition_broadcast) and applied
   to the expert output before accumulation.
"""

import sys

sys.path.insert(0, "/opt/trn_rl_repo/concourse")
sys.path.insert(0, "/opt/trn_rl_repo")

import numpy as np
import ml_dtypes

import concourse.bass as bass
import concourse.tile as tile
from concourse import bacc, mybir
from concourse import bass_utils

F32 = mybir.dt.float32
BF16 = mybir.dt.bfloat16
FP8 = mybir.dt.float8e4
AF = mybir.ActivationFunctionType
OP = mybir.AluOpType

DIM = 384
NE = 8
HID = 4 * DIM  # 1536
NIMG = 4  # images per core
T = NIMG * 1024  # tokens per core
NQ = 3  # channel chunks of 128
NHT = HID // 128  # 12
NCB = 8  # 512-token column blocks
CB = 512
EPS = 1e-6

_cached = None


def _build():
    nc = bacc.Bacc("TRN2", target_bir_lowering=False)

    inp4 = nc.dram_tensor("inp4", [NIMG, DIM, 32, 32], F32, kind="ExternalInput")
    diag = nc.dram_tensor("diag", [NQ, 49, 128, 128], BF16, kind="ExternalInput")
    w1p = nc.dram_tensor("w1p", [NE, 128, 2, HID], FP8, kind="ExternalInput")
    w1c = nc.dram_tensor("w1c", [NE, 128, HID], FP8, kind="ExternalInput")
    w2p = nc.dram_tensor("w2p", [NE, 6, 128, 2, DIM], FP8, kind="ExternalInput")
    b1s = nc.dram_tensor("b1s", [128, NE, NHT], F32, kind="ExternalInput")
    b2s = nc.dram_tensor("b2s", [128, NE, NQ], F32, kind="ExternalInput")
    gws = nc.dram_tensor("gws", [NQ, 128, NE], BF16, kind="ExternalInput")
    chv = nc.dram_tensor("chv", [128, NQ, 4], F32, kind="ExternalInput")
    io8 = nc.dram_tensor("io8", [128, NE], F32, kind="ExternalInput")
    out4 = nc.dram_tensor("out4", [NIMG, DIM, 32, 32], F32, kind="ExternalOutput")

    inp_cm = inp4.rearrange("n c h w -> c n (h w)")  # [384, 4, 1024]
    out_cm = out4.rearrange("n c h w -> c n (h w)")

    with tile.TileContext(nc) as tc:
        # ---------- persistent SBUF ----------
        persist = tc.alloc_tile_pool(name="persist", bufs=1)
        xhat = [persist.tile([128, T], BF16, tag=f"xhat{q}", name=f"xhat{q}") for q in range(NQ)]
        acc = [persist.tile([128, T], BF16, tag=f"acc{q}", name=f"acc{q}") for q in range(NQ)]
        b1t = persist.tile([128, NE, NHT], F32, tag="b1t", name="b1t")
        b2t = persist.tile([128, NE, NQ], F32, tag="b2t", name="b2t")
        gwt = persist.tile([128, NQ, NE], BF16, tag="gwt", name="gwt")
        chvt = persist.tile([128, NQ, 4], F32, tag="chvt", name="chvt")
        io8t = persist.tile([128, NE], F32, tag="io8t", name="io8t")
        onest = persist.tile([128, 128], BF16, tag="onest", name="onest")
        m1v = persist.tile([128, 32], F32, tag="m1v", name="m1v")
        m2v = persist.tile([128, 32], F32, tag="m2v", name="m2v")
        e0v = persist.tile([128, 32], F32, tag="e0v", name="e0v")
        e1v = persist.tile([128, 32], F32, tag="e1v", name="e1v")
        w0v = persist.tile([128, 32], F32, tag="w0v", name="w0v")
        w1v = persist.tile([128, 32], F32, tag="w1v", name="w1v")

        nc.sync.dma_start(b1t[:], b1s[:])
        nc.sync.dma_start(b2t[:], b2s[:])
        nc.sync.dma_start(gwt[:], gws.rearrange("q p e -> p q e"))
        nc.sync.dma_start(chvt[:], chv[:])
        nc.sync.dma_start(io8t[:], io8[:])
        nc.any.memset(onest[:], 1.0)
        xq8a = persist.tile([128, 2, T], FP8, tag="xq8a", name="xq8a")
        xq8b = persist.tile([128, T], FP8, tag="xq8b", name="xq8b")
        epst = persist.tile([128, 1], F32, tag="epst", name="epst")
        nc.any.memset(epst[:], EPS)
        zerot = persist.tile([128, 1], F32, tag="zerot", name="zerot")
        nc.any.memset(zerot[:], 0.0)

        # ---------- phase 1: dwconv + LN stats inputs ----------
        with tc.tile_pool(name="convin", bufs=2) as cpool, \
             tc.tile_pool(name="diagp", bufs=1) as dpool, \
             tc.tile_pool(name="xconv", bufs=1) as xcpool, \
             tc.tile_pool(name="cps", bufs=4, space="PSUM") as cps, \
             tc.tile_pool(name="sps", bufs=2, space="PSUM") as sps, \
             tc.tile_pool(name="lnt", bufs=2) as lnt:
            xconv = [xcpool.tile([128, T], BF16, tag=f"xc{q}", name=f"xc{q}") for q in range(NQ)]
            for q in range(NQ):
                xp = cpool.tile([128, NIMG, 38, 38], BF16, tag="xpad", name="xpad")
                nc.any.memset(xp[:], 0.0)
                for n in range(NIMG):
                    nc.gpsimd.dma_start(
                        xp[:, n, 3:35, 3:35],
                        inp4.rearrange("n c h w -> c n h w")[q * 128:(q + 1) * 128, n],
                    )
                dg = dpool.tile([128, 49, 128], BF16, tag="diag", name="diag")
                nc.sync.dma_start(dg[:], diag.rearrange("q t p m -> p q t m")[:, q])
                for cbg in range(2):  # two groups of 4 column blocks
                    pts = [cps.tile([128, 16, 32], F32, tag="cpsum", name="cpsum") for _ in range(4)]
                    for tap in range(49):
                        dh, dw = tap // 7, tap % 7
                        for j in range(4):
                            cb = cbg * 4 + j
                            n, hh = cb // 2, cb % 2
                            nc.tensor.matmul(
                                pts[j][:],
                                dg[:, tap],
                                xp[:, n, hh * 16 + dh: hh * 16 + dh + 16, dw: dw + 32],
                                start=(tap == 0),
                                stop=(tap == 48),
                            )
                    for j in range(4):
                        cb = cbg * 4 + j
                        sl = slice(cb * CB, (cb + 1) * CB)
                        xcv = xconv[q][:, sl].rearrange("p (a b) -> p a b", a=16)
                        nc.scalar.activation(xcv, pts[j][:], AF.Identity,
                                             bias=chvt[:, q, 0:1], scale=1.0)

            # ---------- phase 2: LN stats + apply ----------
            for cb in range(NCB):
                sl = slice(cb * CB, (cb + 1) * CB)
                pm1 = sps.tile([128, CB], F32, tag="pm1", name="pm1")
                pm2 = sps.tile([128, CB], F32, tag="pm2", name="pm2")
                for q in range(NQ):
                    nc.tensor.matmul(pm1[:], onest[:], xconv[q][:, sl],
                                     start=(q == 0), stop=(q == NQ - 1))
                for q in range(NQ):
                    sqt = lnt.tile([128, CB], BF16, tag="sqt", name="sqt")
                    nc.scalar.activation(sqt[:], xconv[q][:, sl], AF.Square,
                                         bias=zerot[:], scale=1.0)
                    nc.tensor.matmul(pm2[:], onest[:], sqt[:],
                                     start=(q == 0), stop=(q == NQ - 1))
                mus = lnt.tile([128, CB], F32, tag="mus", name="mus")
                nc.vector.tensor_scalar_mul(mus[:], pm1[:], 1.0 / DIM)
                msq = lnt.tile([128, CB], F32, tag="msq", name="msq")
                nc.vector.tensor_tensor(msq[:], mus[:], mus[:], OP.mult)
                var = lnt.tile([128, CB], F32, tag="var", name="var")
                nc.vector.scalar_tensor_tensor(var[:], pm2[:], 1.0 / DIM, msq[:],
                                               OP.mult, OP.subtract)
                sd = lnt.tile([128, CB], F32, tag="sd", name="sd")
                nc.scalar.activation(sd[:], var[:], AF.Sqrt, bias=epst[:], scale=1.0)
                rst = lnt.tile([128, CB], F32, tag="rst", name="rst")
                nc.vector.reciprocal(rst[:], sd[:])
                for q in range(NQ):
                    t1 = lnt.tile([128, CB], F32, tag="t1", name="t1")
                    nc.vector.tensor_tensor(t1[:], xconv[q][:, sl], mus[:],
                                            OP.subtract)
                    t2 = lnt.tile([128, CB], F32, tag="t2", name="t2")
                    nc.vector.tensor_tensor(t2[:], t1[:], rst[:], OP.mult)
                    nc.vector.tensor_scalar(xhat[q][:, sl], t2[:],
                                            chvt[:, q, 1:2], chvt[:, q, 2:3],
                                            OP.mult, OP.add)
                    if q < 2:
                        nc.vector.tensor_copy(xq8a[:, q, sl], xhat[q][:, sl])
                    else:
                        nc.vector.tensor_copy(xq8b[:, sl], xhat[q][:, sl])

        # ---------- phase 3: router logits + top-2 ----------
        with tc.tile_pool(name="lps", bufs=4, space="PSUM") as lps, \
             tc.tile_pool(name="tkt", bufs=6) as tkt:
            for tt in range(32):
                plg = lps.tile([128, NE], F32, tag="plg", name="plg")
                for q in range(NQ):
                    nc.tensor.matmul(plg[:], xhat[q][:, tt * 128:(tt + 1) * 128],
                                     gwt[:, q], start=(q == 0), stop=(q == NQ - 1))
                c1 = slice(tt, tt + 1)
                nc.vector.tensor_reduce(m1v[:, c1], plg[:], mybir.AxisListType.X, OP.max)
                ta = tkt.tile([128, NE], F32, tag="ta", name="ta")
                nc.vector.tensor_scalar(ta[:], plg[:], m1v[:, c1], None, OP.is_equal)
                tb = tkt.tile([128, NE], F32, tag="tb", name="tb")
                nc.vector.tensor_tensor(tb[:], ta[:], io8t[:], OP.mult)
                nc.vector.tensor_reduce(e0v[:, c1], tb[:], mybir.AxisListType.X, OP.max)
                tcm = tkt.tile([128, NE], F32, tag="tc", name="tc")
                nc.vector.scalar_tensor_tensor(tcm[:], ta[:], -1e30, plg[:],
                                               OP.mult, OP.add)
                nc.vector.tensor_reduce(m2v[:, c1], tcm[:], mybir.AxisListType.X, OP.max)
                td = tkt.tile([128, NE], F32, tag="td", name="td")
                nc.vector.tensor_scalar(td[:], tcm[:], m2v[:, c1], None, OP.is_equal)
                te = tkt.tile([128, NE], F32, tag="te", name="te")
                nc.vector.tensor_tensor(te[:], td[:], io8t[:], OP.mult)
                nc.vector.tensor_reduce(e1v[:, c1], te[:], mybir.AxisListType.X, OP.max)
            # softmax over the two top values
            dv = tkt.tile([128, 32], F32, tag="dv", name="dv")
            nc.vector.tensor_tensor(dv[:], m2v[:], m1v[:], OP.subtract)
            ev = tkt.tile([128, 32], F32, tag="ev", name="ev")
            nc.scalar.activation(ev[:], dv[:], AF.Exp, bias=zerot[:], scale=1.0)
            den = tkt.tile([128, 32], F32, tag="den", name="den")
            nc.vector.tensor_scalar_add(den[:], ev[:], 1.0)
            nc.vector.reciprocal(w0v[:], den[:])
            nc.vector.tensor_scalar(w1v[:], w0v[:], -1.0, 1.0, OP.mult, OP.add)

        # ---------- phase 4: per-expert gate broadcast + dense MoE MLP ----------
        with tc.tile_pool(name="wd", bufs=1, space="DRAM") as wdp, \
             tc.tile_pool(name="wtok", bufs=4) as wtp, \
             tc.tile_pool(name="webp", bufs=2) as webp, \
             tc.tile_pool(name="wts", bufs=2) as wts, \
             tc.tile_pool(name="hsb", bufs=13) as hsb, \
             tc.tile_pool(name="hps", bufs=2, space="PSUM") as hps, \
             tc.tile_pool(name="yps", bufs=3, space="PSUM") as yps, \
             tc.tile_pool(name="cmb", bufs=3) as cmb:
            wd = wdp.tile([NE, 32, 128], BF16, name="wd")
            for e in range(NE):
                # gate weight for expert e per token, token-major [tok128, tile32]
                ma = wtp.tile([128, 32], F32, tag="ma", name="ma")
                nc.vector.tensor_scalar(ma[:], e0v[:], float(e), None, OP.is_equal)
                mb = wtp.tile([128, 32], F32, tag="mb", name="mb")
                nc.vector.tensor_tensor(mb[:], ma[:], w0v[:], OP.mult)
                nc.vector.tensor_scalar(ma[:], e1v[:], float(e), None, OP.is_equal)
                mc = wtp.tile([128, 32], F32, tag="mc", name="mc")
                nc.vector.tensor_tensor(mc[:], ma[:], w1v[:], OP.mult)
                wtok = wtp.tile([128, 32], BF16, tag="wtok", name="wtok")
                nc.vector.scalar_tensor_tensor(wtok[:], mb[:], 1.0, mc[:],
                                               OP.mult, OP.add)
                nc.vector.tensor_scalar_mul(wtok[:], wtok[:], 1.0 / 16.0)
                nc.sync.dma_start(wd[e].rearrange("t p -> p t"), wtok[:])
                w1row = webp.tile([1, T], BF16, tag="w1row", name="w1row")
                nc.sync.dma_start(w1row[:], wd[e].rearrange("t p -> () (t p)"))
                web = webp.tile([128, T], BF16, tag="web", name="web")
                nc.gpsimd.partition_broadcast(web[:], w1row[:])

                w1pt = wts.tile([128, 2, HID], FP8, tag="w1pt", name="w1pt")
                nc.sync.dma_start(w1pt[:], w1p[e])
                w1ct = wts.tile([128, HID], FP8, tag="w1ct", name="w1ct")
                nc.sync.dma_start(w1ct[:], w1c[e])
                w2pt = wts.tile([128, 6, 2, DIM], FP8, tag="w2pt", name="w2pt")
                for J in range(6):
                    nc.sync.dma_start(w2pt[:, J], w2p.rearrange("e J p j m -> e J p (j m)")[e, J].rearrange("p x -> p x").rearrange("p (j m) -> p j m", j=2))

                for cb in range(NCB):
                    sl = slice(cb * CB, (cb + 1) * CB)
                    hq8 = [hsb.tile([128, 2, CB], FP8, tag="hq8", name="hq8")
                           for _ in range(6)]
                    for ht in range(NHT):
                        ph = hps.tile([128, CB], F32, tag="ph", name="ph")
                        nc.tensor.matmul(ph[:], w1pt[:, :, ht * 128:(ht + 1) * 128],
                                         xq8a[:, :, sl], start=True, stop=False,
                                         perf_mode=mybir.MatmulPerfMode.DoubleRow)
                        nc.tensor.matmul(ph[:], w1ct[:, ht * 128:(ht + 1) * 128],
                                         xq8b[:, sl], start=False, stop=True)
                        nc.scalar.activation(hq8[ht // 2][:, ht % 2, :], ph[:],
                                             AF.Gelu, bias=b1t[:, e, ht:ht + 1],
                                             scale=1.0 / 16.0)
                    for dq in range(NQ):
                        py = yps.tile([128, CB], F32, tag="py", name="py")
                        for J in range(6):
                            nc.tensor.matmul(py[:],
                                             w2pt[:, J, :, dq * 128:(dq + 1) * 128],
                                             hq8[J][:],
                                             start=(J == 0), stop=(J == 5),
                                             perf_mode=mybir.MatmulPerfMode.DoubleRow)
                        if e == 0:
                            nc.vector.scalar_tensor_tensor(
                                acc[dq][:, sl], py[:], b2t[:, e, dq:dq + 1],
                                web[:, sl], OP.add, OP.mult)
                        else:
                            ytmp = cmb.tile([128, CB], F32, tag="ytmp", name="ytmp")
                            nc.vector.scalar_tensor_tensor(
                                ytmp[:], py[:], b2t[:, e, dq:dq + 1],
                                web[:, sl], OP.add, OP.mult)
                            nc.vector.tensor_tensor(acc[dq][:, sl], acc[dq][:, sl],
                                                    ytmp[:], OP.add)

        # ---------- phase 5: layer-scale + residual + store ----------
        with tc.tile_pool(name="fin", bufs=3) as fin:
            for q in range(NQ):
                res = fin.tile([128, NIMG, 1024], F32, tag="res", name="res")
                nc.sync.dma_start(res[:], inp_cm[q * 128:(q + 1) * 128])
                osb = fin.tile([128, NIMG, 1024], F32, tag="osb", name="osb")
                nc.vector.scalar_tensor_tensor(
                    osb.rearrange("p n x -> p (n x)"), acc[q][:],
                    chvt[:, q, 3:4], res.rearrange("p n x -> p (n x)"),
                    OP.mult, OP.add)
                nc.sync.dma_start(out_cm[q * 128:(q + 1) * 128], osb[:])

        persist.release()

    nc.compile()
    return nc


def _prep(inputs):
    bf = ml_dtypes.bfloat16
    dw_w = np.asarray(inputs["dw_w"], np.float32)  # [384,1,7,7]
    diag = np.zeros((NQ, 49, 128, 128), np.float32)
    ii = np.arange(128)
    for q in range(NQ):
        for tap in range(49):
            diag[q, tap, ii, ii] = dw_w[q * 128:(q + 1) * 128, 0, tap // 7, tap % 7]
    f8 = ml_dtypes.float8_e4m3
    w1 = np.asarray(inputs["w1"], np.float32) * 16.0  # [8,384,1536]
    w2 = np.asarray(inputs["w2"], np.float32) * 16.0  # [8,1536,384]
    w1p = w1[:, :256].reshape(NE, 2, 128, HID).transpose(0, 2, 1, 3)
    w1c = w1[:, 256:]
    w2p = w2.reshape(NE, 6, 2, 128, DIM).transpose(0, 1, 3, 2, 4)
    b1 = np.asarray(inputs["b1"], np.float32)  # [8,1536]
    b2 = np.asarray(inputs["b2"], np.float32)  # [8,384]
    b1s = b1.reshape(NE, NHT, 128).transpose(2, 0, 1)  # [128, 8, 12]
    b2s = 16.0 * b2.reshape(NE, NQ, 128).transpose(2, 0, 1)  # [128, 8, 3]
    gw = np.asarray(inputs["gate_w"], np.float32)  # [8,384]
    gws = gw.reshape(NE, NQ, 128).transpose(1, 2, 0)  # [3,128,8]
    chv = np.stack([
        np.asarray(inputs["dw_b"], np.float32),
        np.asarray(inputs["ln_g"], np.float32),
        np.asarray(inputs["ln_b"], np.float32),
        np.asarray(inputs["layer_scale"], np.float32).reshape(-1),
    ], axis=-1).reshape(NQ, 128, 4).transpose(1, 0, 2)  # [128,3,4]
    io8 = np.broadcast_to(np.arange(NE, dtype=np.float32), (128, NE))
    common = {
        "diag": np.ascontiguousarray(diag.astype(bf)),
        "w1p": np.ascontiguousarray(w1p.astype(f8)),
        "w1c": np.ascontiguousarray(w1c.astype(f8)),
        "w2p": np.ascontiguousarray(w2p.astype(f8)),
        "b1s": np.ascontiguousarray(b1s),
        "b2s": np.ascontiguousarray(b2s),
        "gws": np.ascontiguousarray(gws.astype(bf)),
        "chv": np.ascontiguousarray(chv),
        "io8": np.ascontiguousarray(io8),
    }
    return common


def kernel(**inputs):
    global _cached
    if _cached is None:
        _cached = _build()
    nc = _cached
    common = _prep(inputs)
    inp = np.ascontiguousarray(np.asarray(inputs["input"], np.float32))
    in_maps = []
    for c in range(8):
        m = dict(common)
        m["inp4"] = np.ascontiguousarray(inp[c * NIMG:(c + 1) * NIMG])
        in_maps.append(m)
    res = bass_utils.run_bass_kernel_spmd(nc, in_maps, core_ids=list(range(8)))
    out = np.concatenate([res.results[c]["out4"] for c in range(8)], axis=0)
    return out.astype(np.float32)


if __name__ == "__main__":
    import reference
    inputs = {k: np.asarray(v) for k, v in reference.setup_inputs().items()}
    got = kernel(**inputs)
    exp = np.asarray(reference.reference(**reference.setup_inputs()))
    err = np.abs(got - exp)
    rel = err.max() / np.abs(exp).max()
    print("max abs err:", err.max(), "rel:", rel)


# revision 14
# speedup vs baseline: 7755.9629x; 1.2105x over previous
"""MoE ConvNeXt block (dwconv7x7 -> LN -> top2-of-8 MoE MLP -> layerscale residual)
on 8 trn2 NeuronCores, data-parallel over the batch dim (4 images per core).

Layout strategy: channel-major [C on partitions (3 chunks of 128), tokens on free].
 - dwconv: 49 diagonal-stationary matmuls accumulating in PSUM (per tap, shifted AP
   into an h/w zero-padded input buffer).
 - LN: column sums via ones-stationary matmuls (replicated across partitions), fused
   scale/shift on DVE.
 - router: token-major logits via x-as-stationary matmuls -> [128 tok, 8] PSUM tiles;
   top-2 + softmax with DVE reduce/select ops.
 - MoE: dense (all 8 experts), weight-stationary matmuls; per-expert gate weights
   broadcast across partitions (DRAM bounce + gpsimd partition_broadcast) and applied
   to the expert output before accumulation.
"""

import sys

sys.path.insert(0, "/opt/trn_rl_repo/concourse")
sys.path.insert(0, "/opt/trn_rl_repo")

import numpy as np
import ml_dtypes

import concourse.bass as bass
import concourse.tile as tile
from concourse import bacc, mybir
from concourse import bass_utils

F32 = mybir.dt.float32
BF16 = mybir.dt.bfloat16
FP8 = mybir.dt.float8e4
AF = mybir.ActivationFunctionType
OP = mybir.AluOpType

DIM = 384
NE = 8
HID = 4 * DIM  # 1536
NIMG = 4  # images per core
T = NIMG * 1024  # tokens per core
NQ = 3  # channel chunks of 128
NHT = HID // 128  # 12
NCB = 8  # 512-token column blocks
CB = 512
EPS = 1e-6

_cached = None


def _build():
    nc = bacc.Bacc("TRN2", target_bir_lowering=False)

    inp4 = nc.dram_tensor("inp4", [NIMG, DIM, 32, 32], F32, kind="ExternalInput")
    dgp = nc.dram_tensor("dgp", [NQ, 7, 3, 128, 2, 128], FP8, kind="ExternalInput")
    dgs = nc.dram_tensor("dgs", [NQ, 7, 128, 128], FP8, kind="ExternalInput")
    w1p = nc.dram_tensor("w1p", [NE, 128, 2, HID], FP8, kind="ExternalInput")
    w1c = nc.dram_tensor("w1c", [NE, 128, HID], FP8, kind="ExternalInput")
    w2p = nc.dram_tensor("w2p", [NE, 6, 128, 2, DIM], FP8, kind="ExternalInput")
    b1s = nc.dram_tensor("b1s", [128, NE, NHT], F32, kind="ExternalInput")
    b2s = nc.dram_tensor("b2s", [128, NE, NQ], F32, kind="ExternalInput")
    gws = nc.dram_tensor("gws", [NQ, 128, NE], BF16, kind="ExternalInput")
    chv = nc.dram_tensor("chv", [128, NQ, 4], F32, kind="ExternalInput")
    io8 = nc.dram_tensor("io8", [128, NE], F32, kind="ExternalInput")
    out4 = nc.dram_tensor("out4", [NIMG, DIM, 32, 32], F32, kind="ExternalOutput")

    inp_cm = inp4.rearrange("n c h w -> c n (h w)")  # [384, 4, 1024]
    out_cm = out4.rearrange("n c h w -> c n (h w)")

    with tile.TileContext(nc) as tc:
        # ---------- persistent SBUF ----------
        persist = tc.alloc_tile_pool(name="persist", bufs=1)
        xhat = [persist.tile([128, T], BF16, tag=f"xhat{q}", name=f"xhat{q}") for q in range(NQ)]
        acc = [persist.tile([128, T], BF16, tag=f"acc{q}", name=f"acc{q}") for q in range(NQ)]
        b1t = persist.tile([128, NE, NHT], F32, tag="b1t", name="b1t")
        b2t = persist.tile([128, NE, NQ], F32, tag="b2t", name="b2t")
        gwt = persist.tile([128, NQ, NE], BF16, tag="gwt", name="gwt")
        chvt = persist.tile([128, NQ, 4], F32, tag="chvt", name="chvt")
        io8t = persist.tile([128, NE], F32, tag="io8t", name="io8t")
        onest = persist.tile([128, 128], BF16, tag="onest", name="onest")
        m1v = persist.tile([128, 32], F32, tag="m1v", name="m1v")
        m2v = persist.tile([128, 32], F32, tag="m2v", name="m2v")
        e0v = persist.tile([128, 32], F32, tag="e0v", name="e0v")
        e1v = persist.tile([128, 32], F32, tag="e1v", name="e1v")
        w0v = persist.tile([128, 32], F32, tag="w0v", name="w0v")
        w1v = persist.tile([128, 32], F32, tag="w1v", name="w1v")

        nc.sync.dma_start(b1t[:], b1s[:])
        nc.sync.dma_start(b2t[:], b2s[:])
        nc.sync.dma_start(gwt[:], gws.rearrange("q p e -> p q e"))
        nc.sync.dma_start(chvt[:], chv[:])
        nc.sync.dma_start(io8t[:], io8[:])
        nc.any.memset(onest[:], 1.0)
        xq8a = persist.tile([128, 2, T], FP8, tag="xq8a", name="xq8a")
        xq8b = persist.tile([128, T], FP8, tag="xq8b", name="xq8b")
        epst = persist.tile([128, 1], F32, tag="epst", name="epst")
        nc.any.memset(epst[:], EPS)
        zerot = persist.tile([128, 1], F32, tag="zerot", name="zerot")
        nc.any.memset(zerot[:], 0.0)

        # ---------- phase 1: dwconv + LN stats inputs ----------
        with tc.tile_pool(name="convin", bufs=2) as cpool, \
             tc.tile_pool(name="diagp", bufs=1) as dpool, \
             tc.tile_pool(name="xconv", bufs=1) as xcpool, \
             tc.tile_pool(name="cps", bufs=4, space="PSUM") as cps, \
             tc.tile_pool(name="sps", bufs=2, space="PSUM") as sps, \
             tc.tile_pool(name="lnt", bufs=2) as lnt:
            xconv = [xcpool.tile([128, T], BF16, tag=f"xc{q}", name=f"xc{q}") for q in range(NQ)]
            for q in range(NQ):
                xp = cpool.tile([128, NIMG, 38, 38], BF16, tag="xpad", name="xpad")
                nc.any.memset(xp[:], 0.0)
                for n in range(NIMG):
                    nc.gpsimd.dma_start(
                        xp[:, n, 3:35, 3:35],
                        inp4.rearrange("n c h w -> c n h w")[q * 128:(q + 1) * 128, n],
                    )
                # fp8 copies: slot 0 = padded input, slot 1 = same shifted up one row
                xp8 = cpool.tile([128, 2, NIMG, 38, 38], FP8, tag="xp8", name="xp8")
                nc.any.memset(xp8[:], 0.0)
                nc.vector.tensor_copy(xp8[:, 0], xp[:])
                nc.vector.tensor_copy(xp8[:, 1, :, 0:37, :], xp[:, :, 1:38, :])
                dgpt = dpool.tile([128, 7, 3, 2, 128], FP8, tag="dgpt", name="dgpt")
                nc.sync.dma_start(dgpt[:], dgp.rearrange("q w j p t m -> p q w j t m")[:, q])
                dgst = dpool.tile([128, 7, 128], FP8, tag="dgst", name="dgst")
                nc.sync.dma_start(dgst[:], dgs.rearrange("q w p m -> p q w m")[:, q])
                for cbg in range(2):  # two groups of 4 column blocks
                    pts = [cps.tile([128, 16, 32], F32, tag="cpsum", name="cpsum") for _ in range(4)]
                    for dw in range(7):
                        for jp in range(3):  # dh pairs (0,1),(2,3),(4,5)
                            for j in range(4):
                                cb = cbg * 4 + j
                                n, hh = cb // 2, cb % 2
                                a = hh * 16 + 2 * jp
                                nc.tensor.matmul(
                                    pts[j][:],
                                    dgpt[:, dw, jp],
                                    xp8[:, :, n, a: a + 16, dw: dw + 32],
                                    start=(dw == 0 and jp == 0),
                                    stop=False,
                                    perf_mode=mybir.MatmulPerfMode.DoubleRow,
                                )
                        for j in range(4):  # dh = 6 single tap
                            cb = cbg * 4 + j
                            n, hh = cb // 2, cb % 2
                            nc.tensor.matmul(
                                pts[j][:],
                                dgst[:, dw],
                                xp8[:, 0, n, hh * 16 + 6: hh * 16 + 22, dw: dw + 32],
                                start=False,
                                stop=(dw == 6),
                            )
                    for j in range(4):
                        cb = cbg * 4 + j
                        sl = slice(cb * CB, (cb + 1) * CB)
                        xcv = xconv[q][:, sl].rearrange("p (a b) -> p a b", a=16)
                        nc.scalar.activation(xcv, pts[j][:], AF.Identity,
                                             bias=chvt[:, q, 0:1], scale=1.0 / 16.0)

            # ---------- phase 2: LN stats + apply ----------
            for cb in range(NCB):
                sl = slice(cb * CB, (cb + 1) * CB)
                pm1 = sps.tile([128, CB], F32, tag="pm1", name="pm1")
                pm2 = sps.tile([128, CB], F32, tag="pm2", name="pm2")
                for q in range(NQ):
                    nc.tensor.matmul(pm1[:], onest[:], xconv[q][:, sl],
                                     start=(q == 0), stop=(q == NQ - 1))
                for q in range(NQ):
                    sqt = lnt.tile([128, CB], BF16, tag="sqt", name="sqt")
                    nc.scalar.activation(sqt[:], xconv[q][:, sl], AF.Square,
                                         bias=zerot[:], scale=1.0)
                    nc.tensor.matmul(pm2[:], onest[:], sqt[:],
                                     start=(q == 0), stop=(q == NQ - 1))
                mus = lnt.tile([128, CB], F32, tag="mus", name="mus")
                nc.vector.tensor_scalar_mul(mus[:], pm1[:], 1.0 / DIM)
                msq = lnt.tile([128, CB], F32, tag="msq", name="msq")
                nc.vector.tensor_tensor(msq[:], mus[:], mus[:], OP.mult)
                var = lnt.tile([128, CB], F32, tag="var", name="var")
                nc.vector.scalar_tensor_tensor(var[:], pm2[:], 1.0 / DIM, msq[:],
                                               OP.mult, OP.subtract)
                sd = lnt.tile([128, CB], F32, tag="sd", name="sd")
                nc.scalar.activation(sd[:], var[:], AF.Sqrt, bias=epst[:], scale=1.0)
                rst = lnt.tile([128, CB], F32, tag="rst", name="rst")
                nc.vector.reciprocal(rst[:], sd[:])
                for q in range(NQ):
                    t1 = lnt.tile([128, CB], F32, tag="t1", name="t1")
                    nc.vector.tensor_tensor(t1[:], xconv[q][:, sl], mus[:],
                                            OP.subtract)
                    t2 = lnt.tile([128, CB], F32, tag="t2", name="t2")
                    nc.vector.tensor_tensor(t2[:], t1[:], rst[:], OP.mult)
                    nc.vector.tensor_scalar(xhat[q][:, sl], t2[:],
                                            chvt[:, q, 1:2], chvt[:, q, 2:3],
                                            OP.mult, OP.add)
                    if q < 2:
                        nc.vector.tensor_copy(xq8a[:, q, sl], xhat[q][:, sl])
                    else:
                        nc.vector.tensor_copy(xq8b[:, sl], xhat[q][:, sl])

        # ---------- phase 3: router logits + top-2 ----------
        with tc.tile_pool(name="lps", bufs=4, space="PSUM") as lps, \
             tc.tile_pool(name="tkt", bufs=6) as tkt:
            for tt in range(32):
                plg = lps.tile([128, NE], F32, tag="plg", name="plg")
                for q in range(NQ):
                    nc.tensor.matmul(plg[:], xhat[q][:, tt * 128:(tt + 1) * 128],
                                     gwt[:, q], start=(q == 0), stop=(q == NQ - 1))
                c1 = slice(tt, tt + 1)
                nc.vector.tensor_reduce(m1v[:, c1], plg[:], mybir.AxisListType.X, OP.max)
                ta = tkt.tile([128, NE], F32, tag="ta", name="ta")
                nc.vector.tensor_scalar(ta[:], plg[:], m1v[:, c1], None, OP.is_equal)
                tb = tkt.tile([128, NE], F32, tag="tb", name="tb")
                nc.vector.tensor_tensor(tb[:], ta[:], io8t[:], OP.mult)
                nc.vector.tensor_reduce(e0v[:, c1], tb[:], mybir.AxisListType.X, OP.max)
                tcm = tkt.tile([128, NE], F32, tag="tc", name="tc")
                nc.vector.scalar_tensor_tensor(tcm[:], ta[:], -1e30, plg[:],
                                               OP.mult, OP.add)
                nc.vector.tensor_reduce(m2v[:, c1], tcm[:], mybir.AxisListType.X, OP.max)
                td = tkt.tile([128, NE], F32, tag="td", name="td")
                nc.vector.tensor_scalar(td[:], tcm[:], m2v[:, c1], None, OP.is_equal)
                te = tkt.tile([128, NE], F32, tag="te", name="te")
                nc.vector.tensor_tensor(te[:], td[:], io8t[:], OP.mult)
                nc.vector.tensor_reduce(e1v[:, c1], te[:], mybir.AxisListType.X, OP.max)
            # softmax over the two top values
            dv = tkt.tile([128, 32], F32, tag="dv", name="dv")
            nc.vector.tensor_tensor(dv[:], m2v[:], m1v[:], OP.subtract)
            ev = tkt.tile([128, 32], F32, tag="ev", name="ev")
            nc.scalar.activation(ev[:], dv[:], AF.Exp, bias=zerot[:], scale=1.0)
            den = tkt.tile([128, 32], F32, tag="den", name="den")
            nc.vector.tensor_scalar_add(den[:], ev[:], 1.0)
            nc.vector.reciprocal(w0v[:], den[:])
            nc.vector.tensor_scalar(w1v[:], w0v[:], -1.0, 1.0, OP.mult, OP.add)

        # ---------- phase 4: per-expert gate broadcast + dense MoE MLP ----------
        with tc.tile_pool(name="wd", bufs=1, space="DRAM") as wdp, \
             tc.tile_pool(name="wtok", bufs=4) as wtp, \
             tc.tile_pool(name="webp", bufs=2) as webp, \
             tc.tile_pool(name="wts", bufs=2) as wts, \
             tc.tile_pool(name="hsb", bufs=13) as hsb, \
             tc.tile_pool(name="hps", bufs=2, space="PSUM") as hps, \
             tc.tile_pool(name="yps", bufs=3, space="PSUM") as yps, \
             tc.tile_pool(name="cmb", bufs=3) as cmb:
            wd = wdp.tile([NE, 32, 128], BF16, name="wd")
            for e in range(NE):
                # gate weight for expert e per token, token-major [tok128, tile32]
                ma = wtp.tile([128, 32], F32, tag="ma", name="ma")
                nc.vector.tensor_scalar(ma[:], e0v[:], float(e), None, OP.is_equal)
                mb = wtp.tile([128, 32], F32, tag="mb", name="mb")
                nc.vector.tensor_tensor(mb[:], ma[:], w0v[:], OP.mult)
                nc.vector.tensor_scalar(ma[:], e1v[:], float(e), None, OP.is_equal)
                mc = wtp.tile([128, 32], F32, tag="mc", name="mc")
                nc.vector.tensor_tensor(mc[:], ma[:], w1v[:], OP.mult)
                wtok = wtp.tile([128, 32], BF16, tag="wtok", name="wtok")
                nc.vector.scalar_tensor_tensor(wtok[:], mb[:], 1.0, mc[:],
                                               OP.mult, OP.add)
                nc.vector.tensor_scalar_mul(wtok[:], wtok[:], 1.0 / 16.0)
                nc.sync.dma_start(wd[e].rearrange("t p -> p t"), wtok[:])
                w1row = webp.tile([1, T], BF16, tag="w1row", name="w1row")
                nc.sync.dma_start(w1row[:], wd[e].rearrange("t p -> () (t p)"))
                web = webp.tile([128, T], BF16, tag="web", name="web")
                nc.gpsimd.partition_broadcast(web[:], w1row[:])

                w1pt = wts.tile([128, 2, HID], FP8, tag="w1pt", name="w1pt")
                nc.sync.dma_start(w1pt[:], w1p[e])
                w1ct = wts.tile([128, HID], FP8, tag="w1ct", name="w1ct")
                nc.sync.dma_start(w1ct[:], w1c[e])
                w2pt = wts.tile([128, 6, 2, DIM], FP8, tag="w2pt", name="w2pt")
                for J in range(6):
                    nc.sync.dma_start(w2pt[:, J], w2p.rearrange("e J p j m -> e J p (j m)")[e, J].rearrange("p x -> p x").rearrange("p (j m) -> p j m", j=2))

                for cb in range(NCB):
                    sl = slice(cb * CB, (cb + 1) * CB)
                    hq8 = [hsb.tile([128, 2, CB], FP8, tag="hq8", name="hq8")
                           for _ in range(6)]
                    for ht in range(NHT):
                        ph = hps.tile([128, CB], F32, tag="ph", name="ph")
                        nc.tensor.matmul(ph[:], w1pt[:, :, ht * 128:(ht + 1) * 128],
                                         xq8a[:, :, sl], start=True, stop=False,
                                         perf_mode=mybir.MatmulPerfMode.DoubleRow)
                        nc.tensor.matmul(ph[:], w1ct[:, ht * 128:(ht + 1) * 128],
                                         xq8b[:, sl], start=False, stop=True)
                        nc.scalar.activation(hq8[ht // 2][:, ht % 2, :], ph[:],
                                             AF.Gelu, bias=b1t[:, e, ht:ht + 1],
                                             scale=1.0 / 16.0)
                    for dq in range(NQ):
                        py = yps.tile([128, CB], F32, tag="py", name="py")
                        for J in range(6):
                            nc.tensor.matmul(py[:],
                                             w2pt[:, J, :, dq * 128:(dq + 1) * 128],
                                             hq8[J][:],
                                             start=(J == 0), stop=(J == 5),
                                             perf_mode=mybir.MatmulPerfMode.DoubleRow)
                        if e == 0:
                            nc.vector.scalar_tensor_tensor(
                                acc[dq][:, sl], py[:], b2t[:, e, dq:dq + 1],
                                web[:, sl], OP.add, OP.mult)
                        else:
                            ytmp = cmb.tile([128, CB], F32, tag="ytmp", name="ytmp")
                            nc.vector.scalar_tensor_tensor(
                                ytmp[:], py[:], b2t[:, e, dq:dq + 1],
                                web[:, sl], OP.add, OP.mult)
                            nc.vector.tensor_tensor(acc[dq][:, sl], acc[dq][:, sl],
                                                    ytmp[:], OP.add)

        # ---------- phase 5: layer-scale + residual + store ----------
        with tc.tile_pool(name="fin", bufs=3) as fin:
            for q in range(NQ):
                res = fin.tile([128, NIMG, 1024], F32, tag="res", name="res")
                nc.sync.dma_start(res[:], inp_cm[q * 128:(q + 1) * 128])
                osb = fin.tile([128, NIMG, 1024], F32, tag="osb", name="osb")
                nc.vector.scalar_tensor_tensor(
                    osb.rearrange("p n x -> p (n x)"), acc[q][:],
                    chvt[:, q, 3:4], res.rearrange("p n x -> p (n x)"),
                    OP.mult, OP.add)
                nc.sync.dma_start(out_cm[q * 128:(q + 1) * 128], osb[:])

        persist.release()

    nc.compile()
    return nc


def _prep(inputs):
    bf = ml_dtypes.bfloat16
    f8 = ml_dtypes.float8_e4m3
    dw_w = np.asarray(inputs["dw_w"], np.float32)  # [384,1,7,7]
    dgp = np.zeros((NQ, 7, 3, 128, 2, 128), np.float32)
    dgs = np.zeros((NQ, 7, 128, 128), np.float32)
    ii = np.arange(128)
    for q in range(NQ):
        for dw in range(7):
            for jp in range(3):
                for j in range(2):
                    dgp[q, dw, jp, ii, j, ii] = 16.0 * dw_w[q * 128:(q + 1) * 128, 0, 2 * jp + j, dw]
            dgs[q, dw, ii, ii] = 16.0 * dw_w[q * 128:(q + 1) * 128, 0, 6, dw]
    w1 = np.asarray(inputs["w1"], np.float32) * 16.0  # [8,384,1536]
    w2 = np.asarray(inputs["w2"], np.float32) * 16.0  # [8,1536,384]
    w1p = w1[:, :256].reshape(NE, 2, 128, HID).transpose(0, 2, 1, 3)
    w1c = w1[:, 256:]
    w2p = w2.reshape(NE, 6, 2, 128, DIM).transpose(0, 1, 3, 2, 4)
    b1 = np.asarray(inputs["b1"], np.float32)  # [8,1536]
    b2 = np.asarray(inputs["b2"], np.float32)  # [8,384]
    b1s = b1.reshape(NE, NHT, 128).transpose(2, 0, 1)  # [128, 8, 12]
    b2s = 16.0 * b2.reshape(NE, NQ, 128).transpose(2, 0, 1)  # [128, 8, 3]
    gw = np.asarray(inputs["gate_w"], np.float32)  # [8,384]
    gws = gw.reshape(NE, NQ, 128).transpose(1, 2, 0)  # [3,128,8]
    chv = np.stack([
        np.asarray(inputs["dw_b"], np.float32),
        np.asarray(inputs["ln_g"], np.float32),
        np.asarray(inputs["ln_b"], np.float32),
        np.asarray(inputs["layer_scale"], np.float32).reshape(-1),
    ], axis=-1).reshape(NQ, 128, 4).transpose(1, 0, 2)  # [128,3,4]
    io8 = np.broadcast_to(np.arange(NE, dtype=np.float32), (128, NE))
    common = {
        "dgp": np.ascontiguousarray(dgp.astype(f8)),
        "dgs": np.ascontiguousarray(dgs.astype(f8)),
        "w1p": np.ascontiguousarray(w1p.astype(f8)),
        "w1c": np.ascontiguousarray(w1c.astype(f8)),
        "w2p": np.ascontiguousarray(w2p.astype(f8)),
        "b1s": np.ascontiguousarray(b1s),
        "b2s": np.ascontiguousarray(b2s),
        "gws": np.ascontiguousarray(gws.astype(bf)),
        "chv": np.ascontiguousarray(chv),
        "io8": np.ascontiguousarray(io8),
    }
    return common


def kernel(**inputs):
    global _cached
    if _cached is None:
        _cached = _build()
    nc = _cached
    common = _prep(inputs)
    inp = np.ascontiguousarray(np.asarray(inputs["input"], np.float32))
    in_maps = []
    for c in range(8):
        m = dict(common)
        m["inp4"] = np.ascontiguousarray(inp[c * NIMG:(c + 1) * NIMG])
        in_maps.append(m)
    res = bass_utils.run_bass_kernel_spmd(nc, in_maps, core_ids=list(range(8)))
    out = np.concatenate([res.results[c]["out4"] for c in range(8)], axis=0)
    return out.astype(np.float32)


if __name__ == "__main__":
    import reference
    inputs = {k: np.asarray(v) for k, v in reference.setup_inputs().items()}
    got = kernel(**inputs)
    exp = np.asarray(reference.reference(**reference.setup_inputs()))
    err = np.abs(got - exp)
    rel = err.max() / np.abs(exp).max()
    print("max abs err:", err.max(), "rel:", rel)


# revision 16
# speedup vs baseline: 7897.3325x; 1.0182x over previous
"""MoE ConvNeXt block (dwconv7x7 -> LN -> top2-of-8 MoE MLP -> layerscale residual)
on 8 trn2 NeuronCores, data-parallel over the batch dim (4 images per core).

Layout strategy: channel-major [C on partitions (3 chunks of 128), tokens on free].
 - dwconv: 49 diagonal-stationary matmuls accumulating in PSUM (per tap, shifted AP
   into an h/w zero-padded input buffer).
 - LN: column sums via ones-stationary matmuls (replicated across partitions), fused
   scale/shift on DVE.
 - router: token-major logits via x-as-stationary matmuls -> [128 tok, 8] PSUM tiles;
   top-2 + softmax with DVE reduce/select ops.
 - MoE: dense (all 8 experts), weight-stationary matmuls; per-expert gate weights
   broadcast across partitions (DRAM bounce + gpsimd partition_broadcast) and applied
   to the expert output before accumulation.
"""

import sys

sys.path.insert(0, "/opt/trn_rl_repo/concourse")
sys.path.insert(0, "/opt/trn_rl_repo")

import numpy as np
import ml_dtypes

import concourse.bass as bass
import concourse.tile as tile
from concourse import bacc, mybir
from concourse import bass_utils

F32 = mybir.dt.float32
BF16 = mybir.dt.bfloat16
FP8 = mybir.dt.float8e4
AF = mybir.ActivationFunctionType
OP = mybir.AluOpType

DIM = 384
NE = 8
HID = 4 * DIM  # 1536
NIMG = 4  # images per core
T = NIMG * 1024  # tokens per core
NQ = 3  # channel chunks of 128
NHT = HID // 128  # 12
NCB = 8  # 512-token column blocks
CB = 512
EPS = 1e-6

_cached = None


def _build():
    nc = bacc.Bacc("TRN2", target_bir_lowering=False)

    inp4 = nc.dram_tensor("inp4", [NIMG, DIM, 32, 32], F32, kind="ExternalInput")
    dgp = nc.dram_tensor("dgp", [NQ, 7, 3, 128, 2, 128], FP8, kind="ExternalInput")
    dgs = nc.dram_tensor("dgs", [NQ, 7, 128, 128], FP8, kind="ExternalInput")
    w1p = nc.dram_tensor("w1p", [NE, 128, 2, HID], FP8, kind="ExternalInput")
    w1c = nc.dram_tensor("w1c", [NE, 128, HID], FP8, kind="ExternalInput")
    w2p = nc.dram_tensor("w2p", [NE, 6, 128, 2, DIM], FP8, kind="ExternalInput")
    b1s = nc.dram_tensor("b1s", [128, NE, NHT], F32, kind="ExternalInput")
    b2s = nc.dram_tensor("b2s", [128, NE, NQ], F32, kind="ExternalInput")
    gws = nc.dram_tensor("gws", [NQ, 128, NE], FP8, kind="ExternalInput")
    chv = nc.dram_tensor("chv", [128, NQ, 4], F32, kind="ExternalInput")
    io8 = nc.dram_tensor("io8", [128, NE], F32, kind="ExternalInput")
    out4 = nc.dram_tensor("out4", [NIMG, DIM, 32, 32], F32, kind="ExternalOutput")

    inp_cm = inp4.rearrange("n c h w -> c n (h w)")  # [384, 4, 1024]
    out_cm = out4.rearrange("n c h w -> c n (h w)")

    with tile.TileContext(nc) as tc:
        # ---------- persistent SBUF ----------
        persist = tc.alloc_tile_pool(name="persist", bufs=1)
        acc = [persist.tile([128, T], BF16, tag=f"acc{q}", name=f"acc{q}") for q in range(NQ)]
        b1t = persist.tile([128, NE, NHT], F32, tag="b1t", name="b1t")
        b2t = persist.tile([128, NE, NQ], F32, tag="b2t", name="b2t")
        gwt = persist.tile([128, NQ, NE], FP8, tag="gwt", name="gwt")
        chvt = persist.tile([128, NQ, 4], F32, tag="chvt", name="chvt")
        io8t = persist.tile([128, NE], F32, tag="io8t", name="io8t")
        onest = persist.tile([128, 128], BF16, tag="onest", name="onest")
        m1v = persist.tile([128, 32], F32, tag="m1v", name="m1v")
        m2v = persist.tile([128, 32], F32, tag="m2v", name="m2v")
        e0v = persist.tile([128, 32], F32, tag="e0v", name="e0v")
        e1v = persist.tile([128, 32], F32, tag="e1v", name="e1v")
        w0v = persist.tile([128, 32], F32, tag="w0v", name="w0v")
        w1v = persist.tile([128, 32], F32, tag="w1v", name="w1v")

        nc.sync.dma_start(b1t[:], b1s[:])
        nc.sync.dma_start(b2t[:], b2s[:])
        nc.sync.dma_start(gwt[:], gws.rearrange("q p e -> p q e"))
        nc.sync.dma_start(chvt[:], chv[:])
        nc.sync.dma_start(io8t[:], io8[:])
        nc.any.memset(onest[:], 1.0)
        xq8a = persist.tile([128, 2, T], FP8, tag="xq8a", name="xq8a")
        xq8b = persist.tile([128, T], FP8, tag="xq8b", name="xq8b")
        epst = persist.tile([128, 1], F32, tag="epst", name="epst")
        nc.any.memset(epst[:], EPS)
        zerot = persist.tile([128, 1], F32, tag="zerot", name="zerot")
        nc.any.memset(zerot[:], 0.0)

        # ---------- phase 1: dwconv + LN stats inputs ----------
        with tc.tile_pool(name="convin", bufs=2) as cpool, \
             tc.tile_pool(name="diagp", bufs=1) as dpool, \
             tc.tile_pool(name="xconv", bufs=1) as xcpool, \
             tc.tile_pool(name="cps", bufs=4, space="PSUM") as cps, \
             tc.tile_pool(name="sps", bufs=2, space="PSUM") as sps, \
             tc.tile_pool(name="lnt", bufs=2) as lnt:
            xconv = [xcpool.tile([128, T], BF16, tag=f"xc{q}", name=f"xc{q}") for q in range(NQ)]
            for q in range(NQ):
                # fp8 padded input: slot 0 = rows at +3, slot 1 = same shifted up one row
                xp8 = cpool.tile([128, 2, NIMG, 38, 38], FP8, tag="xp8", name="xp8")
                nc.any.memset(xp8[:], 0.0)
                for n in range(NIMG):
                    src_ap = inp4.rearrange("n c h w -> c n h w")[q * 128:(q + 1) * 128, n]
                    nc.gpsimd.dma_start(xp8[:, 0, n, 3:35, 3:35], src_ap)
                    nc.gpsimd.dma_start(xp8[:, 1, n, 2:34, 3:35], src_ap)
                dgpt = dpool.tile([128, 7, 3, 2, 128], FP8, tag="dgpt", name="dgpt")
                nc.sync.dma_start(dgpt[:], dgp.rearrange("q w j p t m -> p q w j t m")[:, q])
                dgst = dpool.tile([128, 7, 128], FP8, tag="dgst", name="dgst")
                nc.sync.dma_start(dgst[:], dgs.rearrange("q w p m -> p q w m")[:, q])
                for cbg in range(2):  # two groups of 4 column blocks
                    pts = [cps.tile([128, 16, 32], F32, tag="cpsum", name="cpsum") for _ in range(4)]
                    for dw in range(7):
                        for jp in range(3):  # dh pairs (0,1),(2,3),(4,5)
                            for j in range(4):
                                cb = cbg * 4 + j
                                n, hh = cb // 2, cb % 2
                                a = hh * 16 + 2 * jp
                                nc.tensor.matmul(
                                    pts[j][:],
                                    dgpt[:, dw, jp],
                                    xp8[:, :, n, a: a + 16, dw: dw + 32],
                                    start=(dw == 0 and jp == 0),
                                    stop=False,
                                    perf_mode=mybir.MatmulPerfMode.DoubleRow,
                                )
                        for j in range(4):  # dh = 6 single tap
                            cb = cbg * 4 + j
                            n, hh = cb // 2, cb % 2
                            nc.tensor.matmul(
                                pts[j][:],
                                dgst[:, dw],
                                xp8[:, 0, n, hh * 16 + 6: hh * 16 + 22, dw: dw + 32],
                                start=False,
                                stop=(dw == 6),
                            )
                    for j in range(4):
                        cb = cbg * 4 + j
                        sl = slice(cb * CB, (cb + 1) * CB)
                        xcv = xconv[q][:, sl].rearrange("p (a b) -> p a b", a=16)
                        nc.scalar.activation(xcv, pts[j][:], AF.Identity,
                                             bias=chvt[:, q, 0:1], scale=1.0 / 16.0)

            # ---------- phase 2: LN stats + apply ----------
            for cb in range(NCB):
                sl = slice(cb * CB, (cb + 1) * CB)
                pm1 = sps.tile([128, CB], F32, tag="pm1", name="pm1")
                pm2 = sps.tile([128, CB], F32, tag="pm2", name="pm2")
                for q in range(NQ):
                    nc.tensor.matmul(pm1[:], onest[:], xconv[q][:, sl],
                                     start=(q == 0), stop=(q == NQ - 1))
                for q in range(NQ):
                    sqt = lnt.tile([128, CB], BF16, tag="sqt", name="sqt")
                    nc.scalar.activation(sqt[:], xconv[q][:, sl], AF.Square,
                                         bias=zerot[:], scale=1.0)
                    nc.tensor.matmul(pm2[:], onest[:], sqt[:],
                                     start=(q == 0), stop=(q == NQ - 1))
                mus = lnt.tile([128, CB], F32, tag="mus", name="mus")
                nc.vector.tensor_scalar_mul(mus[:], pm1[:], 1.0 / DIM)
                msq = lnt.tile([128, CB], F32, tag="msq", name="msq")
                nc.vector.tensor_tensor(msq[:], mus[:], mus[:], OP.mult)
                var = lnt.tile([128, CB], F32, tag="var", name="var")
                nc.vector.scalar_tensor_tensor(var[:], pm2[:], 1.0 / DIM, msq[:],
                                               OP.mult, OP.subtract)
                sd = lnt.tile([128, CB], F32, tag="sd", name="sd")
                nc.scalar.activation(sd[:], var[:], AF.Sqrt, bias=epst[:], scale=1.0)
                rst = lnt.tile([128, CB], F32, tag="rst", name="rst")
                nc.vector.reciprocal(rst[:], sd[:])
                for q in range(NQ):
                    t1 = lnt.tile([128, CB], F32, tag="t1", name="t1")
                    nc.vector.tensor_tensor(t1[:], xconv[q][:, sl], mus[:],
                                            OP.subtract)
                    t2 = lnt.tile([128, CB], F32, tag="t2", name="t2")
                    nc.vector.tensor_tensor(t2[:], t1[:], rst[:], OP.mult)
                    dst = xq8a[:, q, sl] if q < 2 else xq8b[:, sl]
                    nc.vector.tensor_scalar(dst, t2[:],
                                            chvt[:, q, 1:2], chvt[:, q, 2:3],
                                            OP.mult, OP.add)

        # ---------- phase 3: router logits + top-2 ----------
        with tc.tile_pool(name="lps", bufs=4, space="PSUM") as lps, \
             tc.tile_pool(name="tkt", bufs=6) as tkt:
            for tt in range(32):
                plg = lps.tile([128, NE], F32, tag="plg", name="plg")
                for q in range(NQ):
                    xs = (xq8a[:, q, tt * 128:(tt + 1) * 128] if q < 2
                          else xq8b[:, tt * 128:(tt + 1) * 128])
                    nc.tensor.matmul(plg[:], xs, gwt[:, q],
                                     start=(q == 0), stop=(q == NQ - 1))
                c1 = slice(tt, tt + 1)
                nc.vector.tensor_reduce(m1v[:, c1], plg[:], mybir.AxisListType.X, OP.max)
                ta = tkt.tile([128, NE], F32, tag="ta", name="ta")
                nc.vector.tensor_scalar(ta[:], plg[:], m1v[:, c1], None, OP.is_equal)
                tb = tkt.tile([128, NE], F32, tag="tb", name="tb")
                nc.vector.tensor_tensor(tb[:], ta[:], io8t[:], OP.mult)
                nc.vector.tensor_reduce(e0v[:, c1], tb[:], mybir.AxisListType.X, OP.max)
                tcm = tkt.tile([128, NE], F32, tag="tc", name="tc")
                nc.vector.scalar_tensor_tensor(tcm[:], ta[:], -1e30, plg[:],
                                               OP.mult, OP.add)
                nc.vector.tensor_reduce(m2v[:, c1], tcm[:], mybir.AxisListType.X, OP.max)
                td = tkt.tile([128, NE], F32, tag="td", name="td")
                nc.vector.tensor_scalar(td[:], tcm[:], m2v[:, c1], None, OP.is_equal)
                te = tkt.tile([128, NE], F32, tag="te", name="te")
                nc.vector.tensor_tensor(te[:], td[:], io8t[:], OP.mult)
                nc.vector.tensor_reduce(e1v[:, c1], te[:], mybir.AxisListType.X, OP.max)
            # softmax over the two top values
            dv = tkt.tile([128, 32], F32, tag="dv", name="dv")
            nc.vector.tensor_tensor(dv[:], m2v[:], m1v[:], OP.subtract)
            ev = tkt.tile([128, 32], F32, tag="ev", name="ev")
            nc.scalar.activation(ev[:], dv[:], AF.Exp, bias=zerot[:], scale=1.0)
            den = tkt.tile([128, 32], F32, tag="den", name="den")
            nc.vector.tensor_scalar_add(den[:], ev[:], 1.0)
            nc.vector.reciprocal(w0v[:], den[:])
            nc.vector.tensor_scalar(w1v[:], w0v[:], -1.0, 1.0, OP.mult, OP.add)

        # ---------- phase 4: per-expert gate broadcast + dense MoE MLP ----------
        with tc.tile_pool(name="wd", bufs=1, space="DRAM") as wdp, \
             tc.tile_pool(name="wtok", bufs=4) as wtp, \
             tc.tile_pool(name="webp", bufs=3) as webp, \
             tc.tile_pool(name="wts", bufs=3) as wts, \
             tc.tile_pool(name="hsb", bufs=13) as hsb, \
             tc.tile_pool(name="hps", bufs=2, space="PSUM") as hps, \
             tc.tile_pool(name="yps", bufs=3, space="PSUM") as yps, \
             tc.tile_pool(name="cmb", bufs=3) as cmb:
            wd = wdp.tile([NE, 32, 128], BF16, name="wd")
            for e in range(NE):
                # gate weight for expert e per token, token-major [tok128, tile32]
                ma = wtp.tile([128, 32], F32, tag="ma", name="ma")
                nc.vector.tensor_scalar(ma[:], e0v[:], float(e), None, OP.is_equal)
                mb = wtp.tile([128, 32], F32, tag="mb", name="mb")
                nc.vector.tensor_tensor(mb[:], ma[:], w0v[:], OP.mult)
                nc.vector.tensor_scalar(ma[:], e1v[:], float(e), None, OP.is_equal)
                mc = wtp.tile([128, 32], F32, tag="mc", name="mc")
                nc.vector.tensor_tensor(mc[:], ma[:], w1v[:], OP.mult)
                wtok = wtp.tile([128, 32], BF16, tag="wtok", name="wtok")
                nc.vector.scalar_tensor_tensor(wtok[:], mb[:], 1.0, mc[:],
                                               OP.mult, OP.add)
                nc.vector.tensor_scalar_mul(wtok[:], wtok[:], 1.0 / 16.0)
                nc.sync.dma_start(wd[e].rearrange("t p -> p t"), wtok[:])
                w1row = webp.tile([1, T], BF16, tag="w1row", name="w1row")
                nc.sync.dma_start(w1row[:], wd[e].rearrange("t p -> () (t p)"))
                web = webp.tile([128, T], BF16, tag="web", name="web")
                nc.gpsimd.partition_broadcast(web[:], w1row[:])

                w1pt = wts.tile([128, 2, HID], FP8, tag="w1pt", name="w1pt")
                nc.sync.dma_start(w1pt[:], w1p[e])
                w1ct = wts.tile([128, HID], FP8, tag="w1ct", name="w1ct")
                nc.sync.dma_start(w1ct[:], w1c[e])
                w2pt = wts.tile([128, 6, 2, DIM], FP8, tag="w2pt", name="w2pt")
                for J in range(6):
                    nc.sync.dma_start(w2pt[:, J], w2p.rearrange("e J p j m -> e J p (j m)")[e, J].rearrange("p x -> p x").rearrange("p (j m) -> p j m", j=2))

                for cb in range(NCB):
                    sl = slice(cb * CB, (cb + 1) * CB)
                    hq8 = [hsb.tile([128, 2, CB], FP8, tag="hq8", name="hq8")
                           for _ in range(6)]
                    for ht in range(NHT):
                        ph = hps.tile([128, CB], F32, tag="ph", name="ph")
                        nc.tensor.matmul(ph[:], w1pt[:, :, ht * 128:(ht + 1) * 128],
                                         xq8a[:, :, sl], start=True, stop=False,
                                         perf_mode=mybir.MatmulPerfMode.DoubleRow)
                        nc.tensor.matmul(ph[:], w1ct[:, ht * 128:(ht + 1) * 128],
                                         xq8b[:, sl], start=False, stop=True)
                        nc.scalar.activation(hq8[ht // 2][:, ht % 2, :], ph[:],
                                             AF.Gelu, bias=b1t[:, e, ht:ht + 1],
                                             scale=1.0 / 16.0)
                    for dq in range(NQ):
                        py = yps.tile([128, CB], F32, tag="py", name="py")
                        for J in range(6):
                            nc.tensor.matmul(py[:],
                                             w2pt[:, J, :, dq * 128:(dq + 1) * 128],
                                             hq8[J][:],
                                             start=(J == 0), stop=(J == 5),
                                             perf_mode=mybir.MatmulPerfMode.DoubleRow)
                        if e == 0:
                            nc.vector.scalar_tensor_tensor(
                                acc[dq][:, sl], py[:], b2t[:, e, dq:dq + 1],
                                web[:, sl], OP.add, OP.mult)
                        else:
                            ytmp = cmb.tile([128, CB], F32, tag="ytmp", name="ytmp")
                            nc.vector.scalar_tensor_tensor(
                                ytmp[:], py[:], b2t[:, e, dq:dq + 1],
                                web[:, sl], OP.add, OP.mult)
                            nc.vector.tensor_tensor(acc[dq][:, sl], acc[dq][:, sl],
                                                    ytmp[:], OP.add)

        # ---------- phase 5: layer-scale + residual + store ----------
        with tc.tile_pool(name="fin", bufs=3) as fin:
            for q in range(NQ):
                res = fin.tile([128, NIMG, 1024], F32, tag="res", name="res")
                nc.sync.dma_start(res[:], inp_cm[q * 128:(q + 1) * 128])
                osb = fin.tile([128, NIMG, 1024], F32, tag="osb", name="osb")
                nc.vector.scalar_tensor_tensor(
                    osb.rearrange("p n x -> p (n x)"), acc[q][:],
                    chvt[:, q, 3:4], res.rearrange("p n x -> p (n x)"),
                    OP.mult, OP.add)
                nc.sync.dma_start(out_cm[q * 128:(q + 1) * 128], osb[:])

        persist.release()

    nc.compile()
    return nc


def _prep(inputs):
    bf = ml_dtypes.bfloat16
    f8 = ml_dtypes.float8_e4m3
    dw_w = np.asarray(inputs["dw_w"], np.float32)  # [384,1,7,7]
    dgp = np.zeros((NQ, 7, 3, 128, 2, 128), np.float32)
    dgs = np.zeros((NQ, 7, 128, 128), np.float32)
    ii = np.arange(128)
    for q in range(NQ):
        for dw in range(7):
            for jp in range(3):
                for j in range(2):
                    dgp[q, dw, jp, ii, j, ii] = 16.0 * dw_w[q * 128:(q + 1) * 128, 0, 2 * jp + j, dw]
            dgs[q, dw, ii, ii] = 16.0 * dw_w[q * 128:(q + 1) * 128, 0, 6, dw]
    w1 = np.asarray(inputs["w1"], np.float32) * 16.0  # [8,384,1536]
    w2 = np.asarray(inputs["w2"], np.float32) * 16.0  # [8,1536,384]
    w1p = w1[:, :256].reshape(NE, 2, 128, HID).transpose(0, 2, 1, 3)
    w1c = w1[:, 256:]
    w2p = w2.reshape(NE, 6, 2, 128, DIM).transpose(0, 1, 3, 2, 4)
    b1 = np.asarray(inputs["b1"], np.float32)  # [8,1536]
    b2 = np.asarray(inputs["b2"], np.float32)  # [8,384]
    b1s = b1.reshape(NE, NHT, 128).transpose(2, 0, 1)  # [128, 8, 12]
    b2s = 16.0 * b2.reshape(NE, NQ, 128).transpose(2, 0, 1)  # [128, 8, 3]
    gw = np.asarray(inputs["gate_w"], np.float32)  # [8,384]
    gws = gw.reshape(NE, NQ, 128).transpose(1, 2, 0)  # [3,128,8]
    chv = np.stack([
        np.asarray(inputs["dw_b"], np.float32),
        np.asarray(inputs["ln_g"], np.float32),
        np.asarray(inputs["ln_b"], np.float32),
        np.asarray(inputs["layer_scale"], np.float32).reshape(-1),
    ], axis=-1).reshape(NQ, 128, 4).transpose(1, 0, 2)  # [128,3,4]
    io8 = np.broadcast_to(np.arange(NE, dtype=np.float32), (128, NE))
    common = {
        "dgp": np.ascontiguousarray(dgp.astype(f8)),
        "dgs": np.ascontiguousarray(dgs.astype(f8)),
        "w1p": np.ascontiguousarray(w1p.astype(f8)),
        "w1c": np.ascontiguousarray(w1c.astype(f8)),
        "w2p": np.ascontiguousarray(w2p.astype(f8)),
        "b1s": np.ascontiguousarray(b1s),
        "b2s": np.ascontiguousarray(b2s),
        "gws": np.ascontiguousarray(gws.astype(f8)),
        "chv": np.ascontiguousarray(chv),
        "io8": np.ascontiguousarray(io8),
    }
    return common


def kernel(**inputs):
    global _cached
    if _cached is None:
        _cached = _build()
    nc = _cached
    common = _prep(inputs)
    inp = np.ascontiguousarray(np.asarray(inputs["input"], np.float32))
    in_maps = []
    for c in range(8):
        m = dict(common)
        m["inp4"] = np.ascontiguousarray(inp[c * NIMG:(c + 1) * NIMG])
        in_maps.append(m)
    res = bass_utils.run_bass_kernel_spmd(nc, in_maps, core_ids=list(range(8)))
    out = np.concatenate([res.results[c]["out4"] for c in range(8)], axis=0)
    return out.astype(np.float32)


if __name__ == "__main__":
    import reference
    inputs = {k: np.asarray(v) for k, v in reference.setup_inputs().items()}
    got = kernel(**inputs)
    exp = np.asarray(reference.reference(**reference.setup_inputs()))
    err = np.abs(got - exp)
    rel = err.max() / np.abs(exp).max()
    print("max abs err:", err.max(), "rel:", rel)
